# revision 1
# baseline (speedup 1.0000x reference)
"""Trainium2 Bass kernel for nn_BlockDiagonalLayer.

Computes out[b, n*64+j] = sin(omega[n] * (sum_i x[b,n,i] * W[n,j,i] + bias[n,j]))
for B=2048, N=1024 networks, D_IN=D_OUT=64, sharded over 8 NeuronCores along N.

Device strategy (per core, 128 networks = 64 pairs):
  - lhsT = 128x128 block-diagonal [W_{2p}^T ; W_{2p+1}^T] (fp32, host-packed layout)
  - rhs  = x^T tile [128 (2 nets x 64 i), 512 b] (host-transposed layout, fp32)
  - PE matmul -> PSUM v[j2, b]; small bf16 matmul adds bias (hi/lo split rows)
  - range reduction for sin (ACT table valid only ~[-pi-0.16, pi+0.16]):
      t  = (omega/2pi)*v + MAGIC          (ScalarE Identity; MAGIC rounds to int)
      k2 = (t - MAGIC) * (2pi/omega)      (VectorE tensor_scalar dual op)
      w  = v - k2                         (VectorE tensor_tensor)
      y  = Sin(omega * w)                 (ScalarE, per-partition omega scale)
  - output stored transposed [pair, j2, b]; host transposes back.
Host does layout-only transforms (transpose / block-diag packing / bf16 split).
"""

import numpy as np
import ml_dtypes

import concourse.bass as bass
import concourse.tile as tile
from concourse import bacc, mybir
from concourse.alu_op_type import AluOpType
from concourse.bass_utils import run_bass_kernel_spmd

B, N, D = 2048, 1024, 64
NCORES = 8
NS = N // NCORES          # 128 nets per core
PAIRS = NS // 2           # 64
MMW = 512                 # matmul moving free dim (fp32 max / one PSUM bank)
EW = 1024                 # elementwise tile width (2 PSUM banks)

TWO_PI = float(2.0 * np.pi)
INV_2PI = float(1.0 / (2.0 * np.pi))
MAGIC = float(1.5 * 2 ** 23)

F32 = mybir.dt.float32
BF16 = mybir.dt.bfloat16


def build_bass(repeat: int = 1):
    """Build the per-core Bass program (same NEFF on all 8 cores).

    repeat > 1 re-runs the whole main loop (idempotent writes) for timing.
    """
    nc = bacc.Bacc("TRN2", target_bir_lowering=False, debug=False,
                   num_devices=NCORES)
    xT_d = nc.dram_tensor("xT", [PAIRS, 128, B], F32, kind="ExternalInput")
    w2_d = nc.dram_tensor("w2", [128, PAIRS * 64], F32, kind="ExternalInput")
    b2_d = nc.dram_tensor("b2", [4, PAIRS * 128], BF16, kind="ExternalInput")
    om_d = nc.dram_tensor("om", [128, PAIRS], F32, kind="ExternalInput")
    yT_d = nc.dram_tensor("yT", [PAIRS, 128, B], F32, kind="ExternalOutput")

    with tile.TileContext(nc) as tc:
        with (
            tc.tile_pool(name="aux", bufs=1) as aux_pool,
            tc.tile_pool(name="wconst", bufs=1) as wc_pool,
            tc.tile_pool(name="xin", bufs=4) as x_pool,
            tc.tile_pool(name="oout", bufs=3) as o_pool,
            tc.tile_pool(name="ew", bufs=9) as ew_pool,
            tc.tile_pool(name="ps", bufs=4, space="PSUM") as psum_pool,
        ):
            # --- constants / aux (loaded once) ---
            w2_sb = wc_pool.tile([128, PAIRS * 64], F32)
            for _c in range(8):
                _w = PAIRS * 64 // 8
                nc.scalar.dma_start(w2_sb[:, _c * _w:(_c + 1) * _w],
                                    w2_d[:, _c * _w:(_c + 1) * _w])
            bds = []
            for _i in range(6):
                _bd = wc_pool.tile([128, 128], F32, tag=f"bd{_i}")
                nc.gpsimd.memset(_bd[:], 0.0)
                bds.append(_bd)
            b2_sb = wc_pool.tile([4, PAIRS * 128], BF16)
            nc.gpsimd.dma_start(b2_sb[:], b2_d[:])
            om_sb = aux_pool.tile([128, PAIRS], F32)
            nc.gpsimd.dma_start(om_sb[:], om_d[:])

            magic_sb = aux_pool.tile([128, 1], F32)
            nc.gpsimd.memset(magic_sb[:], MAGIC)
            ones2 = aux_pool.tile([4, MMW], BF16)
            nc.gpsimd.memset(ones2[:], 1.0)

            s1_sb = aux_pool.tile([128, PAIRS], F32)   # omega/2pi
            nc.vector.tensor_scalar_mul(s1_sb[:], om_sb[:], INV_2PI)
            inv_sb = aux_pool.tile([128, PAIRS], F32)  # 1/omega
            nc.vector.reciprocal(inv_sb[:], om_sb[:])
            s2_sb = aux_pool.tile([128, PAIRS], F32)   # 2pi/omega
            nc.vector.tensor_scalar_mul(s2_sb[:], inv_sb[:], TWO_PI)

            # --- main loop (optionally wrapped in a HW loop for timing) ---
            import contextlib
            rep_ctx = tc.For_i(0, repeat, 1) if repeat > 1 else contextlib.nullcontext()
            PB = 2  # pairs batched per DMA transfer
            with rep_ctx:
                for p0 in range(0, PAIRS, PB):
                    xt = x_pool.tile([128, PB * B], F32)
                    nc.sync.dma_start(
                        xt[:].rearrange("p (a b) -> p a b", a=PB),
                        xT_d[p0:p0 + PB].rearrange("a p b -> p a b"))
                    outt = o_pool.tile([128, PB * B], F32)
                    for a in range(PB):
                        p = p0 + a
                        w2t = bds[p % 6][:]
                        nc.vector.tensor_copy(
                            w2t[0:D, 0:D], w2_sb[0:D, p * D:(p + 1) * D])
                        nc.vector.tensor_copy(
                            w2t[D:, D:], w2_sb[D:, p * D:(p + 1) * D])
                        b2t = b2_sb[:, p * 128:(p + 1) * 128]
                        s1p = s1_sb[:, p:p + 1]
                        s2p = s2_sb[:, p:p + 1]
                        omp = om_sb[:, p:p + 1]
                        for e in range(B // EW):
                            v = psum_pool.tile([128, EW], F32)
                            for h in range(EW // MMW):
                                lo = h * MMW
                                bcol = a * B + e * EW + lo
                                nc.tensor.matmul(
                                    v[:, lo:lo + MMW], w2t,
                                    xt[:, bcol:bcol + MMW],
                                    start=True, stop=False)
                                nc.tensor.matmul(
                                    v[:, lo:lo + MMW], b2t, ones2[:],
                                    start=False, stop=True)
                            t = ew_pool.tile([128, EW], F32, tag="ew")
                            nc.scalar.activation(
                                t[:], v[:], mybir.ActivationFunctionType.Identity,
                                bias=magic_sb[:], scale=s1p)
                            k2 = ew_pool.tile([128, EW], F32, tag="ew")
                            nc.vector.tensor_scalar(
                                k2[:], t[:], MAGIC, s2p,
                                op0=AluOpType.subtract, op1=AluOpType.mult)
                            wt = ew_pool.tile([128, EW], F32, tag="ew")
                            nc.vector.tensor_tensor(
                                wt[:], v[:], k2[:], op=AluOpType.subtract)
                            nc.scalar.activation(
                                outt[:, a * B + e * EW:a * B + (e + 1) * EW], wt[:],
                                mybir.ActivationFunctionType.Sin,
                                bias=0.0, scale=omp)
                    nc.scalar.dma_start(
                        yT_d[p0:p0 + PB].rearrange("a p b -> p a b"),
                        outt[:].rearrange("p (a b) -> p a b", a=PB))
    nc.compile()
    return nc


def prep_inputs(x, weights, bias, omega):
    """Host-side layout prep -> list of 8 per-core input dicts."""
    bf16 = ml_dtypes.bfloat16
    x3 = x.reshape(B, NCORES, NS, D)
    # xT_all[c, n, i, b] = x[b, c*128+n, i]; blocked for cache friendliness
    xT_all = np.empty((NCORES, NS, D, B), np.float32)
    BBLK = 128
    for b0 in range(0, B, BBLK):
        xT_all[:, :, :, b0:b0 + BBLK] = x3[b0:b0 + BBLK].transpose(1, 2, 3, 0)

    in_maps = []
    for c in range(NCORES):
        sl = slice(c * NS, (c + 1) * NS)
        wc = weights[sl]                       # [128, 64, 64] (j, i)
        wT = np.ascontiguousarray(wc.transpose(0, 2, 1))  # [net, i, j]
        # dense transposed weights [i2, pair*64 + j]: rows 0-63 = even nets'
        # W^T, rows 64-127 = odd nets' (block-diag assembled on-chip)
        w2_host = np.concatenate([
            wT[0::2].transpose(1, 0, 2).reshape(D, PAIRS * D),
            wT[1::2].transpose(1, 0, 2).reshape(D, PAIRS * D)], axis=0)
        w2_host = np.ascontiguousarray(w2_host)

        bc = bias[sl].astype(np.float32)       # [128, 64]
        b_hi = bc.astype(bf16)
        b_lo = (bc - b_hi.astype(np.float32)).astype(bf16)
        b2 = np.zeros((PAIRS, 4, 128), bf16)
        b2[:, 0, :D] = b_hi[0::2]
        b2[:, 1, :D] = b_lo[0::2]
        b2[:, 2, D:] = b_hi[1::2]
        b2[:, 3, D:] = b_lo[1::2]
        b2_host = np.ascontiguousarray(
            b2.transpose(1, 0, 2).reshape(4, PAIRS * 128))

        oc = omega[sl].astype(np.float32)      # [128]
        om2 = np.repeat(oc.reshape(PAIRS, 2), D, axis=1)   # [64, 128]
        om_host = np.ascontiguousarray(om2.T)              # [128, 64]

        xT_c = np.ascontiguousarray(xT_all[c].reshape(PAIRS, 128, B))
        in_maps.append({"xT": xT_c, "w2": w2_host, "b2": b2_host,
                        "om": om_host})
    return in_maps


def assemble_output(results):
    """[8 cores] of yT [PAIRS, 128, B] -> full [B, N*D]."""
    out = np.empty((B, N * D), np.float32)
    for c in range(NCORES):
        yy = results[c]["yT"].reshape(NS * D, B)
        ov = out[:, c * NS * D:(c + 1) * NS * D]
        for b0 in range(0, B, 128):
            ov[b0:b0 + 128, :] = yy[:, b0:b0 + 128].T
    return out


_NC_CACHE = {}


def kernel(x, weights, bias, omega):
    x = np.ascontiguousarray(x, np.float32)
    weights = np.ascontiguousarray(weights, np.float32)
    bias = np.ascontiguousarray(bias, np.float32)
    omega = np.ascontiguousarray(omega, np.float32)

    if "nc" not in _NC_CACHE:
        _NC_CACHE["nc"] = build_bass()
    nc = _NC_CACHE["nc"]
    in_maps = prep_inputs(x, weights, bias, omega)
    res = run_bass_kernel_spmd(nc, in_maps, core_ids=list(range(NCORES)))
    return assemble_output(res.results)



# revision 2
# speedup vs baseline: 1.5724x; 1.5724x over previous
"""Trainium2 Bass kernel for nn_BlockDiagonalLayer.

Computes out[b, n*64+j] = sin(omega[n] * (sum_i x[b,n,i] * W[n,j,i] + bias[n,j]))
for B=2048, N=1024 networks, D_IN=D_OUT=64, sharded over 8 NeuronCores along N.

Math: with s = (omega/2pi)*y + (omega/2pi)*b, out = sin(2*pi*s) and only
frac(s) matters.  Host prepares NEGATED scaled weights w'' = -(omega/2pi)*W
and bias rows b'' = -(omega/2pi)*b - 768, so the PSUM accumulates
u = -(s + 768) in (-800, -738).  Then (all exact, hardware-verified):
    k = round(u)            via magic constant M15 = 1.5*2^23
    d = k - u = -frac(u) = frac(s) - (integer)   in [-0.5, 0.5]
    y = sin(2*pi*d) = sin(2*pi*s) = reference output.

Precision (max err 5.8e-3 on the real seed-0 inputs, budget 2e-2):
  x quantized to 16-ish bits as xa*2^-8 (fp16, 11-bit int) + xb*2^-9 (fp8e4m3,
  5-bit int residual); the residual matmul uses lhsT fp16 w''*2^-4 against the
  fp8 rhs (mixed-dtype matmul, verified exact on HW).  Weights as fp16 hi+lo
  pair (exact to ~2^-22).  Bias rows as fp16 hi+lo.  Output fp16.

Per core (128 nets = 64 pairs), per pair: 16 matmuls (4 kinds x 4 PSUM
chunks of 512), then range reduction split 3:1 between ACT-path
(Identity(u+M15) on ScalarE + scalar_tensor_tensor on VectorE) and DVE-path
(dual-op tensor_scalar + tensor_tensor), then Sin on ScalarE (fp16 out).
Measured op costs (slope-bench): ACT ~1.45us, DVE ~2.1us per FD=2048 op;
DMA is the roofline (~81 MiB/core).
"""

import numpy as np
import ml_dtypes

import concourse.bass as bass
import concourse.tile as tile
from concourse import bacc, mybir
from concourse.alu_op_type import AluOpType
from concourse.bass_utils import run_bass_kernel_spmd

B, N, D = 2048, 1024, 64
NCORES = 8
NS = N // NCORES          # 128 nets per core
PAIRS = NS // 2           # 64
MMW = 512                 # matmul moving free dim (one PSUM bank)

M15 = float(1.5 * 2.0 ** 23)
TWO_PI = float(2.0 * np.pi)

F32 = mybir.dt.float32
F16 = mybir.dt.float16
F8 = mybir.dt.float8e4

FP8 = ml_dtypes.float8_e4m3fn


def build_bass(repeat: int = 1):
    """Build the per-core Bass program (same NEFF on all 8 cores)."""
    nc = bacc.Bacc("TRN2", target_bir_lowering=False, debug=False,
                   num_devices=NCORES)
    xa_d = nc.dram_tensor("xa", [PAIRS, 128, B], F16, kind="ExternalInput")
    xb_d = nc.dram_tensor("xb", [PAIRS, 128, B], F8, kind="ExternalInput")
    wbd_d = nc.dram_tensor("wbd", [128, PAIRS * 3 * 128], F16,
                           kind="ExternalInput")
    b2_d = nc.dram_tensor("b2", [2, PAIRS * 128], F16, kind="ExternalInput")
    y_d = nc.dram_tensor("y", [PAIRS, 128, B], F16, kind="ExternalOutput")

    with tile.TileContext(nc) as tc:
        with (
            tc.tile_pool(name="aux", bufs=1) as aux_pool,
            tc.tile_pool(name="wconst", bufs=1) as wc_pool,
            tc.tile_pool(name="xa", bufs=3) as xa_pool,
            tc.tile_pool(name="xbp", bufs=3) as xb_pool,
            tc.tile_pool(name="tt", bufs=3) as t_pool,
            tc.tile_pool(name="dd", bufs=3) as d_pool,
            tc.tile_pool(name="yy", bufs=3) as y_pool,
            tc.tile_pool(name="ps", bufs=2, space="PSUM") as psum_pool,
        ):
            # --- constants (loaded once, outside the repeat loop) ---
            wsb = wc_pool.tile([128, PAIRS * 3 * 128], F16)
            wchunk = PAIRS * 3 * 128 // 8
            for c in range(8):
                nc.scalar.dma_start(wsb[:, c * wchunk:(c + 1) * wchunk],
                                    wbd_d[:, c * wchunk:(c + 1) * wchunk])
            b2_sb = wc_pool.tile([2, PAIRS * 128], F16)
            nc.gpsimd.dma_start(b2_sb[:], b2_d[:])
            m15_sb = aux_pool.tile([128, 1], F32)
            nc.gpsimd.memset(m15_sb[:], M15)
            ones2 = aux_pool.tile([2, MMW], F16)
            nc.gpsimd.memset(ones2[:], 1.0)

            import contextlib
            rep_ctx = tc.For_i(0, repeat, 1) if repeat > 1 else contextlib.nullcontext()
            with rep_ctx:
                for p in range(PAIRS):
                    xa = xa_pool.tile([128, B], F16)
                    nc.sync.dma_start(xa[:], xa_d[p])
                    xb = xb_pool.tile([128, B], F8)
                    nc.sync.dma_start(xb[:], xb_d[p])

                    u = psum_pool.tile([128, B], F32, tag="u")
                    base = p * 3 * 128
                    # kind 0/1: w16/wlo x xa; kind 2: wr x xb; then bias rows
                    for k in range(3):
                        wk = wsb[:, base + k * 128: base + (k + 1) * 128]
                        xop = xa if k < 2 else xb
                        for h in range(B // MMW):
                            nc.tensor.matmul(
                                u[:, h * MMW:(h + 1) * MMW], wk,
                                xop[:, h * MMW:(h + 1) * MMW],
                                start=(k == 0), stop=False)
                    bk = b2_sb[:, p * 128:(p + 1) * 128]
                    for h in range(B // MMW):
                        nc.tensor.matmul(
                            u[:, h * MMW:(h + 1) * MMW], bk, ones2[:],
                            start=False, stop=True)

                    d = d_pool.tile([128, B], F32, tag="d")
                    if p % 4 != 3:
                        # ACT path: t = Identity(u + M15) = M15 + round(u)
                        t = t_pool.tile([128, B], F32, tag="t")
                        nc.scalar.activation(
                            t[:], u[:], mybir.ActivationFunctionType.Identity,
                            bias=m15_sb[:], scale=1.0)
                        # d = (t - M15) - u = round(u) - u
                        nc.vector.scalar_tensor_tensor(
                            d[:], t[:], M15, u[:],
                            op0=AluOpType.subtract, op1=AluOpType.subtract)
                    else:
                        # DVE path: k = (u + M15) - M15 = round(u)
                        t = t_pool.tile([128, B], F32, tag="t")
                        nc.vector.tensor_scalar(
                            t[:], u[:], M15, M15,
                            op0=AluOpType.add, op1=AluOpType.subtract)
                        nc.vector.tensor_tensor(
                            d[:], t[:], u[:], op=AluOpType.subtract)

                    y = y_pool.tile([128, B], F16, tag="y")
                    nc.scalar.activation(
                        y[:], d[:], mybir.ActivationFunctionType.Sin,
                        bias=0.0, scale=TWO_PI)
                    nc.gpsimd.dma_start(y_d[p], y[:])
    nc.compile()
    return nc


def prep_inputs(x, weights, bias, omega):
    """Host-side quantization + layout prep -> list of 8 per-core dicts."""
    f16 = np.float16
    om = omega.astype(np.float64)
    scl = om / (2.0 * np.pi)

    # --- negated scaled weights, fp16 hi/lo + fp8-path lhsT (fp16) ---
    wp = -(weights.astype(np.float64) * scl[:, None, None])   # [N, 64(j), 64(i)]
    w16 = wp.astype(f16)
    wlo = (wp - w16.astype(np.float64)).astype(f16)
    wr = (wp * 2.0 ** -4).astype(f16)
    # lhsT orientation: [i, j] = wp[n][j, i]
    w16t = w16.transpose(0, 2, 1)
    wlot = wlo.transpose(0, 2, 1)
    wrt = wr.transpose(0, 2, 1)

    bp = -(bias.astype(np.float64) * scl[:, None]) - 768.0    # [N, 64]
    bhi = bp.astype(f16)
    blo = (bp - bhi.astype(np.float64)).astype(f16)

    # fp8 LUT for residual ints in [-16, 16] scaled 2^-9
    lut = (np.arange(-16, 17, dtype=np.float64) * 2.0 ** -9).astype(FP8)

    in_maps = []
    for c in range(NCORES):
        sl = slice(c * NS, (c + 1) * NS)

        # block-diagonal lhsT per (pair, kind): [PAIRS, 3, 128(i2), 128(j2)]
        bd = np.zeros((PAIRS, 3, 128, 128), f16)
        for k, wt in enumerate((w16t, wlot, wrt)):
            wc = wt[sl]                      # [128, 64, 64] (net, i, j)
            bd[:, k, 0:D, 0:D] = wc[0::2]
            bd[:, k, D:, D:] = wc[1::2]
        wbd = np.ascontiguousarray(
            bd.transpose(2, 0, 1, 3).reshape(128, PAIRS * 3 * 128))

        b2 = np.zeros((2, PAIRS * 128), f16)
        bh, bl = bhi[sl], blo[sl]            # [128, 64]
        b2[0] = np.concatenate(
            [bh[0::2], bh[1::2]], axis=1).reshape(PAIRS, 128).reshape(-1)
        b2[1] = np.concatenate(
            [bl[0::2], bl[1::2]], axis=1).reshape(PAIRS, 128).reshape(-1)

        # x: transpose to [128(i2), B] per pair, then quantize
        xc = x[:, sl, :]                     # [B, 128, 64]
        xT = np.empty((NS, D, B), np.float32)
        BBLK = 128
        for b0 in range(0, B, BBLK):
            xT[:, :, b0:b0 + BBLK] = xc[b0:b0 + BBLK].transpose(1, 2, 0)
        xi = np.rint(xT.astype(np.float64) * 8192.0).astype(np.int32)
        qa = (xi + 16) >> 5                  # 11-bit ints
        qb = xi - (qa << 5)                  # residual in [-16, 16]
        xa16 = (qa.astype(np.float32) * np.float32(2.0 ** -8)).astype(f16)
        xb8 = lut[(qb + 16).reshape(-1)].reshape(qb.shape)
        # [NS, D, B] -> [PAIRS, 2, 64, B] -> [PAIRS, 128, B]
        xa_c = np.ascontiguousarray(
            xa16.reshape(PAIRS, 2, D, B).reshape(PAIRS, 128, B))
        xb_c = np.ascontiguousarray(
            xb8.reshape(PAIRS, 2, D, B).reshape(PAIRS, 128, B))

        in_maps.append({"xa": xa_c, "xb": xb_c, "wbd": wbd, "b2": b2})
    return in_maps


def assemble_output(results):
    """[8 cores] of y [PAIRS, 128, B] fp16 -> full [B, N*D] fp32."""
    out = np.empty((B, N * D), np.float32)
    for c in range(NCORES):
        yy = results[c]["y"].reshape(NS * D, B)
        ov = out[:, c * NS * D:(c + 1) * NS * D]
        for b0 in range(0, B, 128):
            ov[b0:b0 + 128, :] = yy[:, b0:b0 + 128].T.astype(np.float32)
    return out


_NC_CACHE = {}


def kernel(x, weights, bias, omega):
    x = np.ascontiguousarray(x, np.float32)
    weights = np.ascontiguousarray(weights, np.float32)
    bias = np.ascontiguousarray(bias, np.float32)
    omega = np.ascontiguousarray(omega, np.float32)

    if "nc" not in _NC_CACHE:
        _NC_CACHE["nc"] = build_bass()
    nc = _NC_CACHE["nc"]
    in_maps = prep_inputs(x, weights, bias, omega)
    res = run_bass_kernel_spmd(nc, in_maps, core_ids=list(range(NCORES)))
    return assemble_output(res.results)
